# revision 47
# baseline (speedup 1.0000x reference)
"""Trainium2 Bass kernel for AfmoeMoE: token-choice top-2 MoE + shared expert.

Sharding (8 cores):
  - Routed experts: expert-parallel. Core c owns expert c's GLU-MLP weights;
    the host routes tokens (router math replicated bit-exactly on CPU jax),
    gathers each expert's tokens into a fixed-capacity buffer (the
    "all-to-all dispatch"), and scatter-adds results back.
  - Shared expert: tensor-parallel over FS in halves x data-parallel over
    4 token groups. Core c handles token group c//2 with FS-half c%2;
    the two halves' partial outputs are summed on the host.

Per-core device kernel (~10 MB of fp16 weights/activations per invocation):
  - Matmul operands (x, weights, h) are fp16 (e5m10): full PE rate and half
    the HBM bytes of fp32, at ~5.6e-4 output error (the data's dynamic range
    fits e5 easily; bf16 would cost 8x the error for the same bytes, fp32r
    (e8m11) halves the error but doubles the bytes -> 57.5 us vs 40.5 us).
    PSUM accumulation and outputs stay fp32. MM_MODE switches f16/f32r/bf16.
  - One ordered HWDGE input stream (SP ring) sequenced in consumption order:
    per-k (x, Wg) chunks -> Wu chunks -> Wd -> shared weights, so the PE
    starts after the first ~1 MB and weight arrival stays on the critical
    path only once. Outputs go through GPSIMD (SWDGE) to keep the ACT
    sequencer free for silu.
  - Stage 1 runs as a k-outer G pass then U pass (PSUM bank reuse after the
    silu drains G); stage 2 consumes h [F,tok] tiles as lhsT directly, so no
    on-device transposes are needed anywhere.
"""

import math

import numpy as np

B, S, D = 2, 512, 1024
T = B * S
E = 8
F = 768
FS = 768
TOP_K = 2
EPS = 1e-20
ROUTE_SCALE = 1.0
P = 128
N_CORES = 8
SGRP = 256        # shared-expert tokens per core (4 groups x 2 FS-halves)
FSH = FS // 2     # shared-expert intermediate slice per core
PAIR = 512        # token-group size for stage-1 matmuls (rhs free dim)

_compiled = {}
MM_MODE = "f16"


def round_f32r(x):
    """Round fp32 to the PE's fp32r format: e8m11, low 12 mantissa bits zero (RNE)."""
    u = np.ascontiguousarray(x, np.float32).view(np.uint32)
    lsb = (u >> 12) & 1
    u2 = (u + 0x7FF + lsb) & np.uint32(0xFFFFF000)
    return u2.view(np.float32)


def build_nc(cap, repeat=1, act="silu", mm=None):
    if mm is None:
        mm = MM_MODE
    """Build the per-core Bass program (same program on all 8 cores)."""
    import concourse.bacc as bacc
    import concourse.mybir as mybir
    import concourse.tile as tile

    f32 = mybir.dt.float32
    wdt = {"f32r": mybir.dt.float32r, "bf16": mybir.dt.bfloat16,
           "f16": mybir.dt.float16, "f32": mybir.dt.float32}[mm]
    silu = mybir.ActivationFunctionType.Silu
    sigmoid = mybir.ActivationFunctionType.Sigmoid
    KD = D // P    # 8 contraction chunks
    MF = F // P    # 6 expert F-tiles
    MS = FSH // P  # 3 shared F-tiles
    NT = (cap + P - 1) // P  # routed token tiles

    nc = bacc.Bacc("TRN2", target_bir_lowering=False, debug=False,
                   num_devices=N_CORES)

    x_in = nc.dram_tensor("x_in", [D, cap + SGRP], wdt, kind="ExternalInput")
    wv = nc.dram_tensor("wv", [P, NT], f32, kind="ExternalInput")
    wgu_e = nc.dram_tensor("wgu_e", [D, 2 * F], wdt, kind="ExternalInput")
    wd_e = nc.dram_tensor("wd_e", [F, D], wdt, kind="ExternalInput")
    wgu_s = nc.dram_tensor("wgu_s", [D, 2 * FSH], wdt, kind="ExternalInput")
    wd_s = nc.dram_tensor("wd_s", [FSH, D], wdt, kind="ExternalInput")
    r_out = nc.dram_tensor("r_out", [cap, D], f32, kind="ExternalOutput")
    s_out = nc.dram_tensor("s_out", [SGRP, D], f32, kind="ExternalOutput")

    with tile.TileContext(nc) as tc:
        with (
            tc.tile_pool(name="wp", bufs=1) as wp,
            tc.tile_pool(name="dp", bufs=3) as dp,
            tc.tile_pool(name="pp", bufs=2, space="PSUM") as pp,
        ):
            def g_pass(xt, x_off, ntk, wgu, fstride, mf):
                """k-outer G-pass: Gs[m] = Wg_m^T @ x, streaming against DMA."""
                xs = xt[:, :, x_off: x_off + ntk]
                Gs = [pp.tile([P, PAIR], f32, name=f"GU{m}", tag=f"GU{m}", bufs=1)[:, :ntk]
                      for m in range(mf)]
                for kc in range(KD):
                    for m in range(mf):
                        nc.tensor.matmul(Gs[m], wgu[:, kc, m * P:(m + 1) * P],
                                         xs[:, kc, :],
                                         start=(kc == 0), stop=(kc == KD - 1))
                return Gs

            def u_pass(xt, x_off, ntk, wgu, fstride, mf, Gs):
                """silu(G) -> h, then U-pass reusing the G banks, h *= U."""
                xs = xt[:, :, x_off: x_off + ntk]
                h = dp.tile([P, mf, PAIR], wdt, name="h", tag="h", bufs=3)
                for m in range(mf):
                    if act == "silu":
                        nc.scalar.activation(h[:, m, :ntk], Gs[m], silu)
                    else:
                        # CoreSim lacks Silu: silu(G) = G * sigmoid(G)
                        nc.scalar.activation(h[:, m, :ntk], Gs[m], sigmoid)
                        nc.vector.tensor_mul(h[:, m, :ntk], h[:, m, :ntk], Gs[m])
                Us = [pp.tile([P, PAIR], f32, name=f"GU{m}b", tag=f"GU{m}", bufs=1)[:, :ntk]
                      for m in range(mf)]
                for kc in range(KD):
                    for m in range(mf):
                        nc.tensor.matmul(Us[m],
                                         wgu[:, kc, fstride + m * P:fstride + (m + 1) * P],
                                         xs[:, kc, :],
                                         start=(kc == 0), stop=(kc == KD - 1))
                for m in range(mf):
                    nc.vector.tensor_mul(h[:, m, :ntk], h[:, m, :ntk], Us[m])
                return h

            def stage2(h, ntk, tt0, wdt_sb, mf, out_dram, wv_sb):
                """out[tile] = (h_tile^T @ Wd) [* w] -> DRAM, per 128-token tile."""
                ntiles = (ntk + P - 1) // P
                for tp in range(ntiles):
                    th = min(P, ntk - tp * P)
                    tok0 = tp * P
                    tt = tt0 + tp
                    ot = dp.tile([P, D], f32, name="ot", tag="ot", bufs=4)
                    for di in range(D // 512):
                        O = pp.tile([P, 512], f32, name="O", tag="O", bufs=2)
                        for m in range(mf):
                            nc.tensor.matmul(O[:th],
                                             h[:, m, tok0:tok0 + th],
                                             wdt_sb[:, m, di * 512:(di + 1) * 512],
                                             start=(m == 0), stop=(m == mf - 1))
                        if wv_sb is not None:
                            nc.vector.tensor_scalar_mul(ot[:th, di * 512:(di + 1) * 512],
                                                        O[:th], wv_sb[:th, tt:tt + 1])
                        else:
                            nc.vector.tensor_copy(ot[:th, di * 512:(di + 1) * 512], O[:th])
                    nc.gpsimd.dma_start(out=out_dram[tt * P: tt * P + th, :], in_=ot[:th])

            def load_chunked(eng, dram, shape, rearr, name, nchunks):
                """SBUF tile filled by per-chunk DMAs (lets compute start early)."""
                t = wp.tile([P, *shape], wdt, name=name)
                src = dram.ap().rearrange(rearr, p=P)
                if nchunks == 1:
                    eng.dma_start(out=t[:], in_=src[:])
                else:
                    step = shape[0] // nchunks
                    for i in range(nchunks):
                        eng.dma_start(out=t[:, i * step:(i + 1) * step],
                                      in_=src[:, i * step:(i + 1) * step])
                return t

            def input_dmas():
                # One input ring (SP), ordered by consumption time.
                xt = dp.tile([P, KD, cap + SGRP], wdt, name="xt", tag="xt", bufs=2)
                xsrc = x_in.ap().rearrange("(k p) n -> p k n", p=P)
                wgu_sb = wp.tile([P, KD, 2 * F], wdt, name="wgu_sb")
                wgu_src = wgu_e.ap().rearrange("(k p) f -> p k f", p=P)
                for kc in range(KD):
                    nc.sync.dma_start(out=xt[:, kc], in_=xsrc[:, kc])
                    if kc == 0:
                        # split the first chunk so the first matmuls start sooner
                        nc.sync.dma_start(out=wgu_sb[:, 0, :3 * P], in_=wgu_src[:, 0, :3 * P])
                        nc.sync.dma_start(out=wgu_sb[:, 0, 3 * P:F], in_=wgu_src[:, 0, 3 * P:F])
                    else:
                        nc.sync.dma_start(out=wgu_sb[:, kc, :F], in_=wgu_src[:, kc, :F])
                for kc in range(KD):
                    nc.sync.dma_start(out=wgu_sb[:, kc, F:], in_=wgu_src[:, kc, F:])
                wv_sb = wp.tile([P, NT], f32, name="wv_sb")
                nc.sync.dma_start(out=wv_sb[:], in_=wv[:])
                wd_sb = load_chunked(nc.sync, wd_e, [MF, D], "(m p) d -> p m d", "wd_sb", 3)
                wgus_sb = load_chunked(nc.sync, wgu_s, [KD, 2 * FSH], "(k p) f -> p k f", "wgus_sb", 2)
                wds_sb = load_chunked(nc.sync, wd_s, [MS, D], "(m p) d -> p m d", "wds_sb", 1)
                return xt, wgu_sb, wv_sb, wd_sb, wgus_sb, wds_sb

            def body(staged=False):
                xt, wgu_sb, wv_sb, wd_sb, wgus_sb, wds_sb = input_dmas()
                # routed: single stage-1 group when cap <= PAIR (always true for
                # this input); fall back to a grouped loop otherwise.
                if cap <= PAIR:
                    Gs = g_pass(xt, 0, cap, wgu_sb, F, MF)
                    if staged:
                        tc.stage_boundary()
                    h_r = u_pass(xt, 0, cap, wgu_sb, F, MF, Gs)
                    if staged:
                        tc.stage_boundary()
                    stage2(h_r, cap, 0, wd_sb, MF, r_out, wv_sb)
                    if staged:
                        tc.stage_boundary()
                else:
                    for pr in range((cap + PAIR - 1) // PAIR):
                        ntk = min(PAIR, cap - pr * PAIR)
                        Gs = g_pass(xt, pr * PAIR, ntk, wgu_sb, F, MF)
                        h_r = u_pass(xt, pr * PAIR, ntk, wgu_sb, F, MF, Gs)
                        stage2(h_r, ntk, pr * (PAIR // P), wd_sb, MF, r_out, wv_sb)
                Gs_s = g_pass(xt, cap, SGRP, wgus_sb, FSH, MS)
                h_s = u_pass(xt, cap, SGRP, wgus_sb, FSH, MS, Gs_s)
                stage2(h_s, SGRP, 0, wds_sb, MS, s_out, None)

            warm = dp.tile([P, 1], f32, name="warm", tag="warm", bufs=1)
            nc.vector.memset(warm[:], 0.0)
            nc.scalar.activation(warm[:], warm[:],
                                 silu if act == "silu" else sigmoid)

            if repeat == 1:
                body()
            else:
                # unroll 2 bodies per hardware-loop iteration: the Tile
                # scheduler freely pipelines adjacent invocations (stage
                # gates then only bite every other body)
                unroll = 2 if repeat % 2 == 0 else 1
                with tc.For_i(0, repeat // unroll, 1,
                              hint_engines=(mybir.EngineType.PE,),
                              staggered_reset=True):
                    for _ in range(unroll):
                        body(staged=False)

    nc.compile()
    return nc


def _route(x, Wr, bias):
    """Replicate the reference router numerics (jax on CPU)."""
    import jax
    import jax.numpy as jnp

    cpu = jax.devices("cpu")[0]
    with jax.default_device(cpu):
        xj = jax.device_put(np.asarray(x, np.float32), cpu)
        Wj = jax.device_put(np.asarray(Wr, np.float32), cpu)
        bj = jax.device_put(np.asarray(bias, np.float32), cpu)
        logits = xj @ Wj
        scores = jax.nn.sigmoid(logits.astype(jnp.float32))
        _, sel = jax.lax.top_k(scores + bj, TOP_K)
        top = jnp.take_along_axis(scores, sel, axis=1)
        top = top / (top.sum(-1, keepdims=True) + EPS)
        top = top * ROUTE_SCALE
        return np.asarray(sel), np.asarray(top, np.float32)


def prepare(hidden_states, W_gate_router, expert_bias, Wg, Wu, Wd, Wg_s, Wu_s, Wd_s):
    """Host-side routing + sharding. Returns (cap, in_maps, combine_fn)."""
    x = np.ascontiguousarray(np.asarray(hidden_states, np.float32).reshape(T, D))
    sel, wts = _route(x, W_gate_router, expert_bias)

    tok = np.repeat(np.arange(T), TOP_K)
    expf = np.asarray(sel).reshape(-1)
    wf = np.asarray(wts).reshape(-1)
    counts = np.bincount(expf, minlength=E)
    cap = max(256, int(math.ceil(counts.max() / 32)) * 32)
    if cap > PAIR:
        # multi-group: keep 128-granularity and a trailing group of >= 256
        # tokens (N < 256 matmuls run at 1/4 rate in fp32r)
        cap = max(256, int(math.ceil(counts.max() / P)) * P)
        if cap % PAIR == P:
            cap += P

    order = np.argsort(expf, kind="stable")
    starts = np.zeros(E + 1, np.int64)
    starts[1:] = np.cumsum(counts)

    if MM_MODE == "f32r":
        rnd = round_f32r
    elif MM_MODE == "bf16":
        import ml_dtypes
        rnd = lambda a: np.ascontiguousarray(np.asarray(a, np.float32).astype(ml_dtypes.bfloat16))
    elif MM_MODE == "f16":
        rnd = lambda a: np.ascontiguousarray(np.asarray(a, np.float32).astype(np.float16))
    else:
        rnd = lambda a: np.ascontiguousarray(a, np.float32)
    in_maps = []
    toklists = []
    Wg = np.asarray(Wg, np.float32)
    Wu = np.asarray(Wu, np.float32)
    Wd = np.asarray(Wd, np.float32)
    Wg_s = np.asarray(Wg_s, np.float32)
    Wu_s = np.asarray(Wu_s, np.float32)
    Wd_s = np.asarray(Wd_s, np.float32)
    xrnd = rnd(x)
    for c in range(N_CORES):
        g, hh = divmod(c, 2)
        sl = order[starts[c]:starts[c + 1]]
        n_c = counts[c]
        xr = np.zeros((cap, D), xrnd.dtype)
        xr[:n_c] = xrnd[tok[sl]]
        nt_pad = ((cap + P - 1) // P) * P
        wvec = np.zeros((nt_pad,), np.float32)
        wvec[:n_c] = wf[sl]
        toklists.append(tok[sl])
        x_all = np.concatenate([xr, xrnd[g * SGRP:(g + 1) * SGRP]], axis=0)
        in_maps.append({
            "x_in": np.ascontiguousarray(x_all.T),
            "wv": np.ascontiguousarray(wvec.reshape(-1, P).T),
            "wgu_e": rnd(np.concatenate([Wg[c], Wu[c]], axis=1)),
            "wd_e": rnd(Wd[c]),
            "wgu_s": rnd(np.concatenate([Wg_s[:, hh * FSH:(hh + 1) * FSH],
                                         Wu_s[:, hh * FSH:(hh + 1) * FSH]], axis=1)),
            "wd_s": rnd(Wd_s[hh * FSH:(hh + 1) * FSH, :]),
        })

    def combine(results):
        out = np.zeros((T, D), np.float32)
        for c in range(N_CORES):
            g, hh = divmod(c, 2)
            out[g * SGRP:(g + 1) * SGRP] += results[c]["s_out"]
            n_c = counts[c]
            if n_c:
                out[toklists[c]] += results[c]["r_out"][:n_c]
        return out.reshape(B, S, D)

    return cap, in_maps, combine


def kernel(hidden_states, W_gate_router, expert_bias, Wg, Wu, Wd, Wg_s, Wu_s, Wd_s):
    from concourse.bass_utils import run_bass_kernel_spmd

    cap, in_maps, combine = prepare(hidden_states, W_gate_router, expert_bias,
                                    Wg, Wu, Wd, Wg_s, Wu_s, Wd_s)
    nc = _compiled.get(cap)
    if nc is None:
        nc = build_nc(cap)
        _compiled[cap] = nc
    res = run_bass_kernel_spmd(nc, in_maps, core_ids=list(range(N_CORES)))
    out = combine(res.results)
    return out.astype(np.asarray(hidden_states).dtype)


# revision 48
# speedup vs baseline: 1.0688x; 1.0688x over previous
"""Trainium2 Bass kernel for AfmoeMoE: token-choice top-2 MoE + shared expert.

Sharding (8 cores):
  - Routed experts: expert-parallel. Core c owns expert c's GLU-MLP weights;
    the host routes tokens (router math replicated bit-exactly on CPU jax),
    gathers each expert's tokens into a fixed-capacity buffer (the
    "all-to-all dispatch"), and scatter-adds results back.
  - Shared expert: tensor-parallel over FS in halves x data-parallel over
    4 token groups. Core c handles token group c//2 with FS-half c%2;
    the two halves' partial outputs are summed on the host.

Per-core device kernel (~10 MB of fp16 weights/activations per invocation):
  - Matmul operands (x, weights, h) are fp16 (e5m10): full PE rate and half
    the HBM bytes of fp32, at ~5.6e-4 output error (the data's dynamic range
    fits e5 easily; bf16 would cost 8x the error for the same bytes, fp32r
    (e8m11) halves the error but doubles the bytes -> 57.5 us vs 40.5 us).
    PSUM accumulation and outputs stay fp32. MM_MODE switches f16/f32r/bf16.
  - One ordered HWDGE input stream (SP ring) sequenced in consumption order:
    per-k (x, Wg) chunks -> Wu chunks -> Wd -> shared weights, so the PE
    starts after the first ~1 MB and weight arrival stays on the critical
    path only once. Outputs go through GPSIMD (SWDGE) to keep the ACT
    sequencer free for silu.
  - Stage 1 runs as a k-outer G pass then U pass (PSUM bank reuse after the
    silu drains G); stage 2 consumes h [F,tok] tiles as lhsT directly, so no
    on-device transposes are needed anywhere.
"""

import math

import numpy as np

B, S, D = 2, 512, 1024
T = B * S
E = 8
F = 768
FS = 768
TOP_K = 2
EPS = 1e-20
ROUTE_SCALE = 1.0
P = 128
N_CORES = 8
SGRP = 256        # shared-expert tokens per core (4 groups x 2 FS-halves)
FSH = FS // 2     # shared-expert intermediate slice per core
PAIR = 512        # token-group size for stage-1 matmuls (rhs free dim)

_compiled = {}
MM_MODE = "f16"


def round_f32r(x):
    """Round fp32 to the PE's fp32r format: e8m11, low 12 mantissa bits zero (RNE)."""
    u = np.ascontiguousarray(x, np.float32).view(np.uint32)
    lsb = (u >> 12) & 1
    u2 = (u + 0x7FF + lsb) & np.uint32(0xFFFFF000)
    return u2.view(np.float32)


def build_nc(cap, repeat=1, act="silu", mm=None):
    if mm is None:
        mm = MM_MODE
    """Build the per-core Bass program (same program on all 8 cores)."""
    import concourse.bacc as bacc
    import concourse.mybir as mybir
    import concourse.tile as tile

    f32 = mybir.dt.float32
    wdt = {"f32r": mybir.dt.float32r, "bf16": mybir.dt.bfloat16,
           "f16": mybir.dt.float16, "f32": mybir.dt.float32}[mm]
    silu = mybir.ActivationFunctionType.Silu
    sigmoid = mybir.ActivationFunctionType.Sigmoid
    KD = D // P    # 8 contraction chunks
    MF = F // P    # 6 expert F-tiles
    MS = FSH // P  # 3 shared F-tiles
    NT = (cap + P - 1) // P  # routed token tiles

    nc = bacc.Bacc("TRN2", target_bir_lowering=False, debug=False,
                   num_devices=N_CORES)

    x_in = nc.dram_tensor("x_in", [D, cap + SGRP], wdt, kind="ExternalInput")
    wv = nc.dram_tensor("wv", [P, NT], f32, kind="ExternalInput")
    wgu_e = nc.dram_tensor("wgu_e", [D, 2 * F], wdt, kind="ExternalInput")
    wd_e = nc.dram_tensor("wd_e", [F, D], wdt, kind="ExternalInput")
    wgu_s = nc.dram_tensor("wgu_s", [D, 2 * FSH], wdt, kind="ExternalInput")
    wd_s = nc.dram_tensor("wd_s", [FSH, D], wdt, kind="ExternalInput")
    r_out = nc.dram_tensor("r_out", [cap, D], f32, kind="ExternalOutput")
    s_out = nc.dram_tensor("s_out", [SGRP, D], f32, kind="ExternalOutput")

    with tile.TileContext(nc) as tc:
        with (
            tc.tile_pool(name="wp", bufs=1) as wp,
            tc.tile_pool(name="dp", bufs=3) as dp,
            tc.tile_pool(name="pp", bufs=2, space="PSUM") as pp,
        ):
            def g_pass(xt, x_off, ntk, wgu, fstride, mf):
                """k-outer G-pass: Gs[m] = Wg_m^T @ x, streaming against DMA."""
                xs = xt[:, :, x_off: x_off + ntk]
                Gs = [pp.tile([P, PAIR], f32, name=f"GU{m}", tag=f"GU{m}", bufs=1)[:, :ntk]
                      for m in range(mf)]
                for kc in range(KD):
                    for m in range(mf):
                        nc.tensor.matmul(Gs[m], wgu[:, kc, m * P:(m + 1) * P],
                                         xs[:, kc, :],
                                         start=(kc == 0), stop=(kc == KD - 1))
                return Gs

            def u_pass(xt, x_off, ntk, wgu, fstride, mf, Gs):
                """silu(G) -> h, then U-pass reusing the G banks, h *= U."""
                xs = xt[:, :, x_off: x_off + ntk]
                h = dp.tile([P, mf, PAIR], wdt, name="h", tag="h", bufs=3)
                for m in range(mf):
                    if act == "silu":
                        nc.scalar.activation(h[:, m, :ntk], Gs[m], silu)
                    else:
                        # CoreSim lacks Silu: silu(G) = G * sigmoid(G)
                        nc.scalar.activation(h[:, m, :ntk], Gs[m], sigmoid)
                        nc.vector.tensor_mul(h[:, m, :ntk], h[:, m, :ntk], Gs[m])
                Us = [pp.tile([P, PAIR], f32, name=f"GU{m}b", tag=f"GU{m}", bufs=1)[:, :ntk]
                      for m in range(mf)]
                for kc in range(KD):
                    for m in range(mf):
                        nc.tensor.matmul(Us[m],
                                         wgu[:, kc, fstride + m * P:fstride + (m + 1) * P],
                                         xs[:, kc, :],
                                         start=(kc == 0), stop=(kc == KD - 1))
                for m in range(mf):
                    nc.vector.tensor_mul(h[:, m, :ntk], h[:, m, :ntk], Us[m])
                return h

            def stage2(h, ntk, tt0, wdt_sb, mf, out_dram, wv_sb):
                """out[tile] = (h_tile^T @ Wd) [* w] -> DRAM, per 128-token tile."""
                ntiles = (ntk + P - 1) // P
                for tp in range(ntiles):
                    th = min(P, ntk - tp * P)
                    tok0 = tp * P
                    tt = tt0 + tp
                    ot = dp.tile([P, D], f32, name="ot", tag="ot", bufs=4)
                    for di in range(D // 512):
                        O = pp.tile([P, 512], f32, name="O", tag="O", bufs=2)
                        for m in range(mf):
                            nc.tensor.matmul(O[:th],
                                             h[:, m, tok0:tok0 + th],
                                             wdt_sb[:, m, di * 512:(di + 1) * 512],
                                             start=(m == 0), stop=(m == mf - 1))
                        if wv_sb is not None:
                            nc.vector.tensor_scalar_mul(ot[:th, di * 512:(di + 1) * 512],
                                                        O[:th], wv_sb[:th, tt:tt + 1])
                        else:
                            nc.vector.tensor_copy(ot[:th, di * 512:(di + 1) * 512], O[:th])
                    nc.gpsimd.dma_start(out=out_dram[tt * P: tt * P + th, :], in_=ot[:th])

            def load_chunked(eng, dram, shape, rearr, name, nchunks):
                """SBUF tile filled by per-chunk DMAs (lets compute start early)."""
                t = wp.tile([P, *shape], wdt, name=name)
                src = dram.ap().rearrange(rearr, p=P)
                if nchunks == 1:
                    eng.dma_start(out=t[:], in_=src[:])
                else:
                    step = shape[0] // nchunks
                    for i in range(nchunks):
                        eng.dma_start(out=t[:, i * step:(i + 1) * step],
                                      in_=src[:, i * step:(i + 1) * step])
                return t

            def input_dmas():
                # One input ring (SP), ordered by consumption time.
                xt = dp.tile([P, KD, cap + SGRP], wdt, name="xt", tag="xt", bufs=2)
                xsrc = x_in.ap().rearrange("(k p) n -> p k n", p=P)
                wgu_sb = wp.tile([P, KD, 2 * F], wdt, name="wgu_sb")
                wgu_src = wgu_e.ap().rearrange("(k p) f -> p k f", p=P)
                nc.sync.dma_start(out=xt[:, 0], in_=xsrc[:, 0])
                nc.sync.dma_start(out=wgu_sb[:, 0, :F], in_=wgu_src[:, 0, :F])
                for lo in range(1, KD, 2):
                    hi = min(lo + 2, KD)
                    nc.sync.dma_start(out=xt[:, lo:hi], in_=xsrc[:, lo:hi])
                    nc.sync.dma_start(out=wgu_sb[:, lo:hi, :F], in_=wgu_src[:, lo:hi, :F])
                for lo in range(0, KD, 4):
                    nc.sync.dma_start(out=wgu_sb[:, lo:lo + 4, F:], in_=wgu_src[:, lo:lo + 4, F:])
                wv_sb = wp.tile([P, NT], f32, name="wv_sb")
                nc.sync.dma_start(out=wv_sb[:], in_=wv[:])
                wd_sb = load_chunked(nc.sync, wd_e, [MF, D], "(m p) d -> p m d", "wd_sb", 3)
                wgus_sb = load_chunked(nc.sync, wgu_s, [KD, 2 * FSH], "(k p) f -> p k f", "wgus_sb", 2)
                wds_sb = load_chunked(nc.sync, wd_s, [MS, D], "(m p) d -> p m d", "wds_sb", 1)
                return xt, wgu_sb, wv_sb, wd_sb, wgus_sb, wds_sb

            def body(staged=False):
                xt, wgu_sb, wv_sb, wd_sb, wgus_sb, wds_sb = input_dmas()
                # routed: single stage-1 group when cap <= PAIR (always true for
                # this input); fall back to a grouped loop otherwise.
                if cap <= PAIR:
                    Gs = g_pass(xt, 0, cap, wgu_sb, F, MF)
                    if staged:
                        tc.stage_boundary()
                    h_r = u_pass(xt, 0, cap, wgu_sb, F, MF, Gs)
                    if staged:
                        tc.stage_boundary()
                    stage2(h_r, cap, 0, wd_sb, MF, r_out, wv_sb)
                    if staged:
                        tc.stage_boundary()
                else:
                    for pr in range((cap + PAIR - 1) // PAIR):
                        ntk = min(PAIR, cap - pr * PAIR)
                        Gs = g_pass(xt, pr * PAIR, ntk, wgu_sb, F, MF)
                        h_r = u_pass(xt, pr * PAIR, ntk, wgu_sb, F, MF, Gs)
                        stage2(h_r, ntk, pr * (PAIR // P), wd_sb, MF, r_out, wv_sb)
                Gs_s = g_pass(xt, cap, SGRP, wgus_sb, FSH, MS)
                h_s = u_pass(xt, cap, SGRP, wgus_sb, FSH, MS, Gs_s)
                stage2(h_s, SGRP, 0, wds_sb, MS, s_out, None)

            warm = dp.tile([P, 1], f32, name="warm", tag="warm", bufs=1)
            nc.vector.memset(warm[:], 0.0)
            nc.scalar.activation(warm[:], warm[:],
                                 silu if act == "silu" else sigmoid)

            if repeat == 1:
                body()
            else:
                # unroll 2 bodies per hardware-loop iteration: the Tile
                # scheduler freely pipelines adjacent invocations (stage
                # gates then only bite every other body)
                unroll = 2 if repeat % 2 == 0 else 1
                with tc.For_i(0, repeat // unroll, 1,
                              hint_engines=(mybir.EngineType.PE,),
                              staggered_reset=True):
                    for _ in range(unroll):
                        body(staged=False)

    nc.compile()
    return nc


def _route(x, Wr, bias):
    """Replicate the reference router numerics (jax on CPU)."""
    import jax
    import jax.numpy as jnp

    cpu = jax.devices("cpu")[0]
    with jax.default_device(cpu):
        xj = jax.device_put(np.asarray(x, np.float32), cpu)
        Wj = jax.device_put(np.asarray(Wr, np.float32), cpu)
        bj = jax.device_put(np.asarray(bias, np.float32), cpu)
        logits = xj @ Wj
        scores = jax.nn.sigmoid(logits.astype(jnp.float32))
        _, sel = jax.lax.top_k(scores + bj, TOP_K)
        top = jnp.take_along_axis(scores, sel, axis=1)
        top = top / (top.sum(-1, keepdims=True) + EPS)
        top = top * ROUTE_SCALE
        return np.asarray(sel), np.asarray(top, np.float32)


def prepare(hidden_states, W_gate_router, expert_bias, Wg, Wu, Wd, Wg_s, Wu_s, Wd_s):
    """Host-side routing + sharding. Returns (cap, in_maps, combine_fn)."""
    x = np.ascontiguousarray(np.asarray(hidden_states, np.float32).reshape(T, D))
    sel, wts = _route(x, W_gate_router, expert_bias)

    tok = np.repeat(np.arange(T), TOP_K)
    expf = np.asarray(sel).reshape(-1)
    wf = np.asarray(wts).reshape(-1)
    counts = np.bincount(expf, minlength=E)
    cap = max(256, int(math.ceil(counts.max() / 32)) * 32)
    if cap > PAIR:
        # multi-group: keep 128-granularity and a trailing group of >= 256
        # tokens (N < 256 matmuls run at 1/4 rate in fp32r)
        cap = max(256, int(math.ceil(counts.max() / P)) * P)
        if cap % PAIR == P:
            cap += P

    order = np.argsort(expf, kind="stable")
    starts = np.zeros(E + 1, np.int64)
    starts[1:] = np.cumsum(counts)

    if MM_MODE == "f32r":
        rnd = round_f32r
    elif MM_MODE == "bf16":
        import ml_dtypes
        rnd = lambda a: np.ascontiguousarray(np.asarray(a, np.float32).astype(ml_dtypes.bfloat16))
    elif MM_MODE == "f16":
        rnd = lambda a: np.ascontiguousarray(np.asarray(a, np.float32).astype(np.float16))
    else:
        rnd = lambda a: np.ascontiguousarray(a, np.float32)
    in_maps = []
    toklists = []
    Wg = np.asarray(Wg, np.float32)
    Wu = np.asarray(Wu, np.float32)
    Wd = np.asarray(Wd, np.float32)
    Wg_s = np.asarray(Wg_s, np.float32)
    Wu_s = np.asarray(Wu_s, np.float32)
    Wd_s = np.asarray(Wd_s, np.float32)
    xrnd = rnd(x)
    for c in range(N_CORES):
        g, hh = divmod(c, 2)
        sl = order[starts[c]:starts[c + 1]]
        n_c = counts[c]
        xr = np.zeros((cap, D), xrnd.dtype)
        xr[:n_c] = xrnd[tok[sl]]
        nt_pad = ((cap + P - 1) // P) * P
        wvec = np.zeros((nt_pad,), np.float32)
        wvec[:n_c] = wf[sl]
        toklists.append(tok[sl])
        x_all = np.concatenate([xr, xrnd[g * SGRP:(g + 1) * SGRP]], axis=0)
        in_maps.append({
            "x_in": np.ascontiguousarray(x_all.T),
            "wv": np.ascontiguousarray(wvec.reshape(-1, P).T),
            "wgu_e": rnd(np.concatenate([Wg[c], Wu[c]], axis=1)),
            "wd_e": rnd(Wd[c]),
            "wgu_s": rnd(np.concatenate([Wg_s[:, hh * FSH:(hh + 1) * FSH],
                                         Wu_s[:, hh * FSH:(hh + 1) * FSH]], axis=1)),
            "wd_s": rnd(Wd_s[hh * FSH:(hh + 1) * FSH, :]),
        })

    def combine(results):
        out = np.zeros((T, D), np.float32)
        for c in range(N_CORES):
            g, hh = divmod(c, 2)
            out[g * SGRP:(g + 1) * SGRP] += results[c]["s_out"]
            n_c = counts[c]
            if n_c:
                out[toklists[c]] += results[c]["r_out"][:n_c]
        return out.reshape(B, S, D)

    return cap, in_maps, combine


def kernel(hidden_states, W_gate_router, expert_bias, Wg, Wu, Wd, Wg_s, Wu_s, Wd_s):
    from concourse.bass_utils import run_bass_kernel_spmd

    cap, in_maps, combine = prepare(hidden_states, W_gate_router, expert_bias,
                                    Wg, Wu, Wd, Wg_s, Wu_s, Wd_s)
    nc = _compiled.get(cap)
    if nc is None:
        nc = build_nc(cap)
        _compiled[cap] = nc
    res = run_bass_kernel_spmd(nc, in_maps, core_ids=list(range(N_CORES)))
    out = combine(res.results)
    return out.astype(np.asarray(hidden_states).dtype)


# revision 49
# speedup vs baseline: 1.1126x; 1.0410x over previous
"""Trainium2 Bass kernel for AfmoeMoE: token-choice top-2 MoE + shared expert.

Sharding (8 cores):
  - Routed experts: expert-parallel. Core c owns expert c's GLU-MLP weights;
    the host routes tokens (router math replicated bit-exactly on CPU jax),
    gathers each expert's tokens into a fixed-capacity buffer (the
    "all-to-all dispatch"), and scatter-adds results back.
  - Shared expert: tensor-parallel over FS in halves x data-parallel over
    4 token groups. Core c handles token group c//2 with FS-half c%2;
    the two halves' partial outputs are summed on the host.

Per-core device kernel (~10 MB of fp16 weights/activations per invocation):
  - Matmul operands (x, weights, h) are fp16 (e5m10): full PE rate and half
    the HBM bytes of fp32, at ~5.6e-4 output error (the data's dynamic range
    fits e5 easily; bf16 would cost 8x the error for the same bytes, fp32r
    (e8m11) halves the error but doubles the bytes -> 57.5 us vs 40.5 us).
    PSUM accumulation and outputs stay fp32. MM_MODE switches f16/f32r/bf16.
  - One ordered HWDGE input stream (SP ring) sequenced in consumption order:
    per-k (x, Wg) chunks -> Wu chunks -> Wd -> shared weights, so the PE
    starts after the first ~1 MB and weight arrival stays on the critical
    path only once. Outputs go through GPSIMD (SWDGE) to keep the ACT
    sequencer free for silu.
  - Stage 1 runs as a k-outer G pass then U pass (PSUM bank reuse after the
    silu drains G); stage 2 consumes h [F,tok] tiles as lhsT directly, so no
    on-device transposes are needed anywhere.
"""

import math

import numpy as np

B, S, D = 2, 512, 1024
T = B * S
E = 8
F = 768
FS = 768
TOP_K = 2
EPS = 1e-20
ROUTE_SCALE = 1.0
P = 128
N_CORES = 8
SGRP = 256        # shared-expert tokens per core (4 groups x 2 FS-halves)
FSH = FS // 2     # shared-expert intermediate slice per core
PAIR = 512        # token-group size for stage-1 matmuls (rhs free dim)

_compiled = {}
MM_MODE = "f16"


def round_f32r(x):
    """Round fp32 to the PE's fp32r format: e8m11, low 12 mantissa bits zero (RNE)."""
    u = np.ascontiguousarray(x, np.float32).view(np.uint32)
    lsb = (u >> 12) & 1
    u2 = (u + 0x7FF + lsb) & np.uint32(0xFFFFF000)
    return u2.view(np.float32)


def build_nc(cap, repeat=1, act="silu", mm=None):
    if mm is None:
        mm = MM_MODE
    """Build the per-core Bass program (same program on all 8 cores)."""
    import concourse.bacc as bacc
    import concourse.mybir as mybir
    import concourse.tile as tile

    f32 = mybir.dt.float32
    wdt = {"f32r": mybir.dt.float32r, "bf16": mybir.dt.bfloat16,
           "f16": mybir.dt.float16, "f32": mybir.dt.float32}[mm]
    silu = mybir.ActivationFunctionType.Silu
    sigmoid = mybir.ActivationFunctionType.Sigmoid
    KD = D // P    # 8 contraction chunks
    MF = F // P    # 6 expert F-tiles
    MS = FSH // P  # 3 shared F-tiles
    NT = (cap + P - 1) // P  # routed token tiles

    nc = bacc.Bacc("TRN2", target_bir_lowering=False, debug=False,
                   num_devices=N_CORES)

    x_in = nc.dram_tensor("x_in", [D, cap + SGRP], wdt, kind="ExternalInput")
    wv = nc.dram_tensor("wv", [P, NT], f32, kind="ExternalInput")
    wgu_e = nc.dram_tensor("wgu_e", [D, 2 * F], wdt, kind="ExternalInput")
    wd_e = nc.dram_tensor("wd_e", [F, D], wdt, kind="ExternalInput")
    wgu_s = nc.dram_tensor("wgu_s", [D, 2 * FSH], wdt, kind="ExternalInput")
    wd_s = nc.dram_tensor("wd_s", [FSH, D], wdt, kind="ExternalInput")
    r_out = nc.dram_tensor("r_out", [cap, D], f32, kind="ExternalOutput")
    s_out = nc.dram_tensor("s_out", [SGRP, D], f32, kind="ExternalOutput")

    with tile.TileContext(nc) as tc:
        with (
            tc.tile_pool(name="wp", bufs=1) as wp,
            tc.tile_pool(name="dp", bufs=3) as dp,
            tc.tile_pool(name="pp", bufs=2, space="PSUM") as pp,
        ):
            def g_pass(xt, x_off, ntk, wgu, fstride, mf):
                """k-outer G-pass: Gs[m] = Wg_m^T @ x, streaming against DMA."""
                xs = xt[:, :, x_off: x_off + ntk]
                Gs = [pp.tile([P, PAIR], f32, name=f"GU{m}", tag=f"GU{m}", bufs=1)[:, :ntk]
                      for m in range(mf)]
                for kc in range(KD):
                    for m in range(mf):
                        nc.tensor.matmul(Gs[m], wgu[:, kc, m * P:(m + 1) * P],
                                         xs[:, kc, :],
                                         start=(kc == 0), stop=(kc == KD - 1))
                return Gs

            def u_pass(xt, x_off, ntk, wgu, fstride, mf, Gs):
                """silu(G) -> h, then U-pass reusing the G banks, h *= U."""
                xs = xt[:, :, x_off: x_off + ntk]
                h = dp.tile([P, mf, PAIR], wdt, name="h", tag="h", bufs=3)
                for m in range(mf):
                    if act == "silu":
                        nc.scalar.activation(h[:, m, :ntk], Gs[m], silu)
                    else:
                        # CoreSim lacks Silu: silu(G) = G * sigmoid(G)
                        nc.scalar.activation(h[:, m, :ntk], Gs[m], sigmoid)
                        nc.vector.tensor_mul(h[:, m, :ntk], h[:, m, :ntk], Gs[m])
                Us = [pp.tile([P, PAIR], f32, name=f"GU{m}b", tag=f"GU{m}", bufs=1)[:, :ntk]
                      for m in range(mf)]
                for kc in range(KD):
                    for m in range(mf):
                        nc.tensor.matmul(Us[m],
                                         wgu[:, kc, fstride + m * P:fstride + (m + 1) * P],
                                         xs[:, kc, :],
                                         start=(kc == 0), stop=(kc == KD - 1))
                for m in range(mf):
                    nc.vector.tensor_mul(h[:, m, :ntk], h[:, m, :ntk], Us[m])
                return h

            def stage2(h, ntk, tt0, wdt_sb, mf, out_dram, wv_sb):
                """out[tile] = (h_tile^T @ Wd) [* w] -> DRAM, per 128-token tile."""
                ntiles = (ntk + P - 1) // P
                for tp in range(ntiles):
                    th = min(P, ntk - tp * P)
                    tok0 = tp * P
                    tt = tt0 + tp
                    ot = dp.tile([P, D], f32, name="ot", tag="ot", bufs=4)
                    for di in range(D // 512):
                        O = pp.tile([P, 512], f32, name="O", tag="O", bufs=2)
                        for m in range(mf):
                            nc.tensor.matmul(O[:th],
                                             h[:, m, tok0:tok0 + th],
                                             wdt_sb[:, m, di * 512:(di + 1) * 512],
                                             start=(m == 0), stop=(m == mf - 1))
                        if wv_sb is not None:
                            nc.vector.tensor_scalar_mul(ot[:th, di * 512:(di + 1) * 512],
                                                        O[:th], wv_sb[:th, tt:tt + 1])
                        else:
                            nc.vector.tensor_copy(ot[:th, di * 512:(di + 1) * 512], O[:th])
                    nc.gpsimd.dma_start(out=out_dram[tt * P: tt * P + th, :], in_=ot[:th])

            def load_chunked(eng, dram, shape, rearr, name, nchunks):
                """SBUF tile filled by per-chunk DMAs (lets compute start early)."""
                t = wp.tile([P, *shape], wdt, name=name)
                src = dram.ap().rearrange(rearr, p=P)
                if nchunks == 1:
                    eng.dma_start(out=t[:], in_=src[:])
                else:
                    step = shape[0] // nchunks
                    for i in range(nchunks):
                        eng.dma_start(out=t[:, i * step:(i + 1) * step],
                                      in_=src[:, i * step:(i + 1) * step])
                return t

            def input_dmas():
                # One input ring (SP), ordered by consumption time.
                xt = dp.tile([P, KD, cap + SGRP], wdt, name="xt", tag="xt", bufs=2)
                xsrc = x_in.ap().rearrange("(k p) n -> p k n", p=P)
                wgu_sb = wp.tile([P, KD, 2 * F], wdt, name="wgu_sb")
                wgu_src = wgu_e.ap().rearrange("(k p) f -> p k f", p=P)
                nc.sync.dma_start(out=xt[:, 0], in_=xsrc[:, 0])
                nc.sync.dma_start(out=wgu_sb[:, 0, :F], in_=wgu_src[:, 0, :F])
                for lo in range(1, KD, 2):
                    hi = min(lo + 2, KD)
                    nc.sync.dma_start(out=xt[:, lo:hi], in_=xsrc[:, lo:hi])
                    nc.sync.dma_start(out=wgu_sb[:, lo:hi, :F], in_=wgu_src[:, lo:hi, :F])
                for lo in range(0, KD, 4):
                    nc.sync.dma_start(out=wgu_sb[:, lo:lo + 4, F:], in_=wgu_src[:, lo:lo + 4, F:])
                wv_sb = wp.tile([P, NT], f32, name="wv_sb")
                nc.sync.dma_start(out=wv_sb[:], in_=wv[:])
                wd_sb = load_chunked(nc.sync, wd_e, [MF, D], "(m p) d -> p m d", "wd_sb", 3)
                wgus_sb = load_chunked(nc.sync, wgu_s, [KD, 2 * FSH], "(k p) f -> p k f", "wgus_sb", 2)
                wds_sb = load_chunked(nc.sync, wd_s, [MS, D], "(m p) d -> p m d", "wds_sb", 1)
                return xt, wgu_sb, wv_sb, wd_sb, wgus_sb, wds_sb

            def body(staged=False):
                xt, wgu_sb, wv_sb, wd_sb, wgus_sb, wds_sb = input_dmas()
                # routed: single stage-1 group when cap <= PAIR (always true for
                # this input); fall back to a grouped loop otherwise.
                if cap <= PAIR:
                    Gs = g_pass(xt, 0, cap, wgu_sb, F, MF)
                    if staged:
                        tc.stage_boundary()
                    h_r = u_pass(xt, 0, cap, wgu_sb, F, MF, Gs)
                    if staged:
                        tc.stage_boundary()
                    stage2(h_r, cap, 0, wd_sb, MF, r_out, wv_sb)
                    if staged:
                        tc.stage_boundary()
                else:
                    for pr in range((cap + PAIR - 1) // PAIR):
                        ntk = min(PAIR, cap - pr * PAIR)
                        Gs = g_pass(xt, pr * PAIR, ntk, wgu_sb, F, MF)
                        h_r = u_pass(xt, pr * PAIR, ntk, wgu_sb, F, MF, Gs)
                        stage2(h_r, ntk, pr * (PAIR // P), wd_sb, MF, r_out, wv_sb)
                Gs_s = g_pass(xt, cap, SGRP, wgus_sb, FSH, MS)
                h_s = u_pass(xt, cap, SGRP, wgus_sb, FSH, MS, Gs_s)
                stage2(h_s, SGRP, 0, wds_sb, MS, s_out, None)

            warm = dp.tile([P, 1], f32, name="warm", tag="warm", bufs=1)
            nc.vector.memset(warm[:], 0.0)
            nc.scalar.activation(warm[:], warm[:],
                                 silu if act == "silu" else sigmoid)

            if repeat == 1:
                body()
            else:
                # unroll 2 bodies per hardware-loop iteration: the Tile
                # scheduler freely pipelines adjacent invocations (stage
                # gates then only bite every other body)
                unroll = 2 if repeat % 2 == 0 else 1
                with tc.For_i(0, repeat // unroll, 1,
                              hint_engines=(mybir.EngineType.PE,),
                              staggered_reset=True):
                    for _ in range(unroll):
                        body(staged=False)

    nc.compile()
    return nc


def _route(x, Wr, bias):
    """Replicate the reference router numerics (jax on CPU)."""
    import jax
    import jax.numpy as jnp

    cpu = jax.devices("cpu")[0]
    with jax.default_device(cpu):
        xj = jax.device_put(np.asarray(x, np.float32), cpu)
        Wj = jax.device_put(np.asarray(Wr, np.float32), cpu)
        bj = jax.device_put(np.asarray(bias, np.float32), cpu)
        logits = xj @ Wj
        scores = jax.nn.sigmoid(logits.astype(jnp.float32))
        _, sel = jax.lax.top_k(scores + bj, TOP_K)
        top = jnp.take_along_axis(scores, sel, axis=1)
        top = top / (top.sum(-1, keepdims=True) + EPS)
        top = top * ROUTE_SCALE
        return np.asarray(sel), np.asarray(top, np.float32)


def prepare(hidden_states, W_gate_router, expert_bias, Wg, Wu, Wd, Wg_s, Wu_s, Wd_s):
    """Host-side routing + sharding. Returns (cap, in_maps, combine_fn)."""
    x = np.ascontiguousarray(np.asarray(hidden_states, np.float32).reshape(T, D))
    sel, wts = _route(x, W_gate_router, expert_bias)

    tok = np.repeat(np.arange(T), TOP_K)
    expf = np.asarray(sel).reshape(-1)
    wf = np.asarray(wts).reshape(-1)
    counts = np.bincount(expf, minlength=E)
    cap = max(256, int(math.ceil(counts.max() / 8)) * 8)
    if cap > PAIR:
        # multi-group: keep 128-granularity and a trailing group of >= 256
        # tokens (N < 256 matmuls run at 1/4 rate in fp32r)
        cap = max(256, int(math.ceil(counts.max() / P)) * P)
        if cap % PAIR == P:
            cap += P

    order = np.argsort(expf, kind="stable")
    starts = np.zeros(E + 1, np.int64)
    starts[1:] = np.cumsum(counts)

    if MM_MODE == "f32r":
        rnd = round_f32r
    elif MM_MODE == "bf16":
        import ml_dtypes
        rnd = lambda a: np.ascontiguousarray(np.asarray(a, np.float32).astype(ml_dtypes.bfloat16))
    elif MM_MODE == "f16":
        rnd = lambda a: np.ascontiguousarray(np.asarray(a, np.float32).astype(np.float16))
    else:
        rnd = lambda a: np.ascontiguousarray(a, np.float32)
    in_maps = []
    toklists = []
    Wg = np.asarray(Wg, np.float32)
    Wu = np.asarray(Wu, np.float32)
    Wd = np.asarray(Wd, np.float32)
    Wg_s = np.asarray(Wg_s, np.float32)
    Wu_s = np.asarray(Wu_s, np.float32)
    Wd_s = np.asarray(Wd_s, np.float32)
    xrnd = rnd(x)
    for c in range(N_CORES):
        g, hh = divmod(c, 2)
        sl = order[starts[c]:starts[c + 1]]
        n_c = counts[c]
        xr = np.zeros((cap, D), xrnd.dtype)
        xr[:n_c] = xrnd[tok[sl]]
        nt_pad = ((cap + P - 1) // P) * P
        wvec = np.zeros((nt_pad,), np.float32)
        wvec[:n_c] = wf[sl]
        toklists.append(tok[sl])
        x_all = np.concatenate([xr, xrnd[g * SGRP:(g + 1) * SGRP]], axis=0)
        in_maps.append({
            "x_in": np.ascontiguousarray(x_all.T),
            "wv": np.ascontiguousarray(wvec.reshape(-1, P).T),
            "wgu_e": rnd(np.concatenate([Wg[c], Wu[c]], axis=1)),
            "wd_e": rnd(Wd[c]),
            "wgu_s": rnd(np.concatenate([Wg_s[:, hh * FSH:(hh + 1) * FSH],
                                         Wu_s[:, hh * FSH:(hh + 1) * FSH]], axis=1)),
            "wd_s": rnd(Wd_s[hh * FSH:(hh + 1) * FSH, :]),
        })

    def combine(results):
        out = np.zeros((T, D), np.float32)
        for c in range(N_CORES):
            g, hh = divmod(c, 2)
            out[g * SGRP:(g + 1) * SGRP] += results[c]["s_out"]
            n_c = counts[c]
            if n_c:
                out[toklists[c]] += results[c]["r_out"][:n_c]
        return out.reshape(B, S, D)

    return cap, in_maps, combine


def kernel(hidden_states, W_gate_router, expert_bias, Wg, Wu, Wd, Wg_s, Wu_s, Wd_s):
    from concourse.bass_utils import run_bass_kernel_spmd

    cap, in_maps, combine = prepare(hidden_states, W_gate_router, expert_bias,
                                    Wg, Wu, Wd, Wg_s, Wu_s, Wd_s)
    nc = _compiled.get(cap)
    if nc is None:
        nc = build_nc(cap)
        _compiled[cap] = nc
    res = run_bass_kernel_spmd(nc, in_maps, core_ids=list(range(N_CORES)))
    out = combine(res.results)
    return out.astype(np.asarray(hidden_states).dtype)
